# revision 13
# baseline (speedup 1.0000x reference)
"""Trainium2 Bass kernel for the CRule loss.

Math (identical to the reference, restructured):
    Hs = H @ y_pred.T                       # [C, B]
    loss[b] = (sum_c Hs[c,b] - y_pred[b,:] @ Hs[:,b]) / sum(H)
            = (y_pred[b,:] . colsum(H) - y_pred[b,:] @ H @ y_pred[b,:]^T) / sum(H)

Rewriting with  Z = y_pred @ H  and  colsum(H)[k] = sum_c H[c,k]:
    loss[b] = -(1/sumH) * sum_k y[b,k] * (Z[b,k] - colsum[k])

Implementation per 128-row batch tile:
  - y is host-padded from 1000 to 1024 contraction columns (bf16); the two
    last padding columns are set to 1.0.  H is host-padded to 1024 rows of
    zeros; on device the kernel writes (-colsum) into those two rows as a
    bf16 value + bf16 residual pair.  The plain matmul  ypad @ Hpad  then
    accumulates  Z - colsum_bcast  directly in PSUM - no extra rank-1 ops.
  - The stationary operand (transposed y) comes from one hardware
    DMA-transpose per tile (3D out AP [128, 8, 128]) on the scalar-engine
    HWDGE ring; natural-layout y tiles load on the sync ring.
  - One scalar_tensor_tensor per 500-column half computes
    sum_k (-1/sumH)*y*(Z-colsum) fused on the vector engine (the
    per-partition scalar slot carries -1/sumH); a vector add of the two
    halves writes the loss column.

y_true is unused by the reference and therefore ignored.

Sharding: data-parallel over the batch dim across 8 cores (2048 rows each),
H replicated in every core's SBUF. No collectives.

Precision: bf16 inputs, fp32 PSUM accumulation; ~3e-5 relative error vs the
fp32 reference (the loss statistic is insensitive to bf16 rounding because
E[Z] ~ colsum/2 cancels the first-order error).
"""

import os

import numpy as np
import ml_dtypes

import concourse.bass as bass
import concourse.mybir as mybir
from concourse import bacc
from concourse.bass_utils import run_bass_kernel_spmd
from concourse.tile import TileContext

B = 16384
C = 1000
CP = 1024            # padded contraction dim
N_CORES = 8
B_SH = B // N_CORES  # 2048 rows per core
P = 128
NB = B_SH // P       # 16 batch tiles per core
CK = CP // P         # 8 contraction chunks of 128
KN = 2               # output-column halves
KNS = C // KN        # 500 (fits one PSUM bank in fp32)

F32 = mybir.dt.float32
BF16 = mybir.dt.bfloat16
MULT = mybir.AluOpType.mult
ADD = mybir.AluOpType.add
SUB = mybir.AluOpType.subtract

_CACHE = {}
LAST_RESULTS = None


def _build():
    nc = bacc.Bacc()
    y = nc.dram_tensor("y", [B_SH, CP], BF16, kind="ExternalInput")
    h = nc.dram_tensor("h", [CP, C], BF16, kind="ExternalInput")
    out = nc.dram_tensor("loss_out", [P, NB], F32, kind="ExternalOutput")

    with TileContext(nc) as tc:
        with (
            tc.tile_pool(name="const", bufs=1) as constp,
            tc.tile_pool(name="hpool", bufs=1) as hp,
            tc.tile_pool(name="ypool", bufs=3) as yp,
            tc.tile_pool(name="ytpool", bufs=3) as ytp,
            tc.tile_pool(name="scr", bufs=2) as scrp,
            tc.tile_pool(name="accs", bufs=4) as accp,
            tc.tile_pool(name="pr", bufs=1, space="PSUM") as prp,
            tc.tile_pool(name="pz", bufs=4, space="PSUM") as pzp,
        ):
            ones_cf = constp.tile([P, 1], F32)
            nc.gpsimd.memset(ones_cf, 1.0)
            ones_rf = constp.tile([1, P], F32)
            nc.gpsimd.memset(ones_rf, 1.0)
            ones_col = constp.tile([P, 1], BF16)
            nc.vector.tensor_copy(ones_col, ones_cf)

            # H chunks: h_sb[:, ck*C:(ck+1)*C] = H[ck*128:(ck+1)*128, :]
            # single DMA (3D AP) so consumers wait on one semaphore
            h_sb = hp.tile([P, CK * C], BF16)
            h_re = h[:, :].rearrange("(ck p) k -> p ck k", p=P)
            nc.sync.dma_start(
                out=h_sb.rearrange("p (ck k) -> p ck k", ck=CK),
                in_=h_re,
            )

            # colsum(H)[k] = sum_c H[c,k]  -> [1, C]
            cs_f = constp.tile([1, C], F32)
            for kn in range(KN):
                cs_ps = pzp.tile([1, KNS], F32, tag="z", name=f"cs_ps{kn}")
                for ck in range(CK):
                    nc.tensor.matmul(
                        cs_ps,
                        lhsT=ones_col,
                        rhs=h_sb[:, ck * C + kn * KNS: ck * C + (kn + 1) * KNS],
                        start=(ck == 0),
                        stop=(ck == CK - 1),
                    )
                nc.vector.tensor_copy(cs_f[:, kn * KNS:(kn + 1) * KNS], cs_ps)

            # write -colsum into H's zero padding rows 1022/1023 (chunk 7,
            # partitions 126/127) as bf16 value + bf16 residual, so the
            # padded matmul accumulates Z - colsum_bcast directly.
            cs_neg = constp.tile([1, C], F32)
            nc.vector.tensor_scalar_mul(cs_neg, cs_f, -1.0)
            r0_t = constp.tile([1, C], BF16)
            nc.vector.tensor_copy(r0_t, cs_neg)                  # r0 = bf16(-cs)
            res_f = constp.tile([1, C], F32)
            nc.vector.tensor_tensor(res_f, cs_neg, r0_t, op=SUB)  # -cs - r0
            res_t = constp.tile([1, C], BF16)
            nc.vector.tensor_copy(res_t, res_f)
            # engines can't address partitions 126/127 directly; DMA can
            nc.sync.dma_start(out=h_sb[P - 2:P - 1, (CK - 1) * C:CK * C], in_=r0_t)
            nc.sync.dma_start(out=h_sb[P - 1:P, (CK - 1) * C:CK * C], in_=res_t)

            # -1/sumH broadcast across partitions
            sum_h = constp.tile([1, 1], F32)
            nc.vector.reduce_sum(sum_h, cs_f, axis=mybir.AxisListType.X)
            recip_f = constp.tile([1, 1], F32)
            nc.vector.reciprocal(recip_f, sum_h)
            nrecip = constp.tile([1, 1], F32)
            nc.vector.tensor_scalar_mul(nrecip, recip_f, -1.0)
            recip_ps = prp.tile([P, 1], F32, name="recip_ps")
            nc.tensor.matmul(recip_ps, lhsT=ones_rf, rhs=nrecip, start=True, stop=True)
            nrecip_bc = constp.tile([P, 1], F32)
            nc.vector.tensor_copy(nrecip_bc, recip_ps)

            loss_acc = constp.tile([P, NB], F32)

            for i in range(NB):
                y_tile = yp.tile([P, CP], BF16, name="y_tile")
                nc.sync.dma_start(out=y_tile, in_=y[i * P:(i + 1) * P, :])

                # hardware transpose: yt[p, ck, b] = y[i*128+b, ck*128+p]
                yt = ytp.tile([P, CK * P], BF16, name="yt")
                nc.scalar.dma_start_transpose(
                    out=yt.rearrange("p (ck b) -> p ck b", ck=CK),
                    in_=y[i * P:(i + 1) * P, :],
                )

                # PSUM <- ypad @ Hpad = Z - colsum_bcast
                s_half = []
                for kn in range(KN):
                    pz = pzp.tile([P, KNS], F32, tag="z", name="pz")
                    for ck in range(CK):
                        nc.tensor.matmul(
                            pz,
                            lhsT=yt[:, ck * P:(ck + 1) * P],
                            rhs=h_sb[:, ck * C + kn * KNS: ck * C + (kn + 1) * KNS],
                            start=(ck == 0),
                            stop=(ck == CK - 1),
                        )
                    # s_kn = sum_k (-1/sumH) * (Z-colsum) * y   (fused)
                    s_o = accp.tile([P, 1], F32, name="s_o")
                    scr = scrp.tile([P, KNS], F32, name="scr")
                    nc.vector.scalar_tensor_tensor(
                        out=scr,
                        in0=pz,
                        scalar=nrecip_bc,
                        in1=y_tile[:, kn * KNS:(kn + 1) * KNS],
                        op0=MULT,
                        op1=MULT,
                        accum_out=s_o,
                    )
                    s_half.append(s_o)

                nc.vector.tensor_add(loss_acc[:, i:i + 1], s_half[0], s_half[1])

            nc.sync.dma_start(out=out[:, :], in_=loss_acc)

    if not nc.is_finalized():
        nc.finalize()
    return nc


def kernel(**inputs):
    global LAST_RESULTS
    y_pred = np.asarray(inputs["y_pred"])
    H = np.asarray(inputs["H"])
    assert y_pred.shape == (B, C) and H.shape == (C, C)

    # host-side layout/dtype prep: bf16 cast + contraction-dim zero-pad.
    # The last two padding columns of y are 1.0: they multiply the (-colsum)
    # rows the kernel writes into H's padding.
    y_b = np.zeros((B, CP), dtype=ml_dtypes.bfloat16)
    y_b[:, :C] = y_pred.astype(ml_dtypes.bfloat16)
    y_b[:, CP - 2:] = 1.0
    h_b = np.zeros((CP, C), dtype=ml_dtypes.bfloat16)
    h_b[:C, :] = H.astype(ml_dtypes.bfloat16)

    nc = _CACHE.get("nc")
    if nc is None:
        nc = _build()
        _CACHE["nc"] = nc

    in_maps = [
        {"y": np.ascontiguousarray(y_b[s * B_SH:(s + 1) * B_SH]), "h": h_b}
        for s in range(N_CORES)
    ]
    res = run_bass_kernel_spmd(
        nc,
        in_maps,
        core_ids=list(range(N_CORES)),
        trace=bool(int(os.environ.get("KBENCH_TRACE", "0"))),
    )
    LAST_RESULTS = res
    # loss_out is [128, 16] partition-major: element [p, i] = loss for shard
    # row i*128 + p. Transpose+flatten restores batch order per shard.
    loss = np.concatenate(
        [np.asarray(r["loss_out"]).T.reshape(-1) for r in res.results]
    ).astype(np.float32)
    return loss


# revision 16
# speedup vs baseline: 1.4488x; 1.4488x over previous
"""Trainium2 Bass kernel for the CRule loss.

Math (identical to the reference, restructured):
    Hs = H @ y_pred.T                       # [C, B]
    loss[b] = (sum_c Hs[c,b] - y_pred[b,:] @ Hs[:,b]) / sum(H)
            = (y_pred[b,:] . colsum(H) - y_pred[b,:] @ H @ y_pred[b,:]^T) / sum(H)

Rewriting with  Z = y_pred @ H  and  colsum(H)[k] = sum_c H[c,k]:
    loss[b] = -(1/sumH) * sum_k y[b,k] * (Z[b,k] - colsum[k])

Kernel structure (per core, 2048 batch rows):
  - Inputs arrive as bf16 in two layouts prepared on the host: natural
    [2048, 1024] (contraction dim zero-padded, last two pad columns = 1.0)
    and transposed [1024, 2048].  H is host-padded to 1024 rows.
  - Everything is loaded into resident SBUF with a few large plain DMAs
    (no device-side transposes -> a single DMA xbar mode, no mode-switch
    serialization).
  - The kernel computes colsum(H) with ones^T @ H matmuls, writes
    (-colsum) as a bf16 value+residual pair into H's two zero padding rows
    (via DMA; compute engines can't address partitions 126/127), so the
    plain padded matmul  ypad @ Hpad  accumulates  Z - colsum_bcast
    directly in PSUM.
  - Per 128-row tile: 16 K=128 matmuls (stationary = transposed-y slices),
    then one scalar_tensor_tensor per 500-column half computes
    sum_k (-1/sumH) * y * (Z-colsum) fused on the vector engine (the
    per-partition scalar slot carries -1/sumH); a vector add of the two
    halves writes the loss column.

y_true is unused by the reference and therefore ignored.

Sharding: data-parallel over the batch dim across 8 cores, H replicated in
every core's SBUF. No collectives.

Precision: bf16 inputs, fp32 PSUM accumulation, fp32 colsum correction;
~2e-5 relative error vs the fp32 reference (the loss statistic is
insensitive to bf16 rounding: E[Z] ~ colsum/2 cancels first-order error).
"""

import os

import numpy as np
import ml_dtypes

import concourse.bass as bass
import concourse.mybir as mybir
from concourse import bacc
from concourse.bass_utils import run_bass_kernel_spmd
from concourse.tile import TileContext

B = 16384
C = 1000
CP = 1024            # padded contraction dim
N_CORES = 8
B_SH = B // N_CORES  # 2048 rows per core
P = 128
NB = B_SH // P       # 16 batch tiles per core
CK = CP // P         # 8 contraction chunks of 128
KN = 2               # output-column halves
KNS = C // KN        # 500 (fits one PSUM bank in fp32)
NBLK = 4             # input streaming blocks (4 tiles each)

F32 = mybir.dt.float32
BF16 = mybir.dt.bfloat16
MULT = mybir.AluOpType.mult
ADD = mybir.AluOpType.add
SUB = mybir.AluOpType.subtract

_CACHE = {}
LAST_RESULTS = None


def _build():
    nc = bacc.Bacc()
    y = nc.dram_tensor("y", [B_SH, CP], BF16, kind="ExternalInput")
    yt_d = nc.dram_tensor("yt", [CP, B_SH], BF16, kind="ExternalInput")
    h = nc.dram_tensor("h", [CP, C], BF16, kind="ExternalInput")
    out = nc.dram_tensor("loss_out", [P, NB], F32, kind="ExternalOutput")

    with TileContext(nc) as tc:
        with (
            tc.tile_pool(name="const", bufs=1) as constp,
            tc.tile_pool(name="big", bufs=1) as bigp,
            tc.tile_pool(name="scr", bufs=2) as scrp,
            tc.tile_pool(name="accs", bufs=4) as accp,
            tc.tile_pool(name="pr", bufs=1, space="PSUM") as prp,
            tc.tile_pool(name="pz", bufs=4, space="PSUM") as pzp,
        ):
            ones_cf = constp.tile([P, 1], F32)
            nc.gpsimd.memset(ones_cf, 1.0)
            ones_rf = constp.tile([1, P], F32)
            nc.gpsimd.memset(ones_rf, 1.0)
            ones_col = constp.tile([P, 1], BF16)
            nc.vector.tensor_copy(ones_col, ones_cf)

            # H chunks: h_sb[:, ck*C:(ck+1)*C] = H[ck*128:(ck+1)*128, :]
            h_sb = bigp.tile([P, CK * C], BF16)
            nc.sync.dma_start(
                out=h_sb.rearrange("p (ck k) -> p ck k", ck=CK),
                in_=h[:, :].rearrange("(ck p) k -> p ck k", p=P),
            )

            # resident transposed y: yt_sb[p, ck, b] = y[b, ck*128+p]
            # loaded in NBLK batch blocks so the PE can start early
            yt_sb = bigp.tile([P, CK * B_SH], BF16)
            yt_re = yt_sb.rearrange("p (ck b) -> p ck b", ck=CK)
            yt_src = yt_d[:, :].rearrange("(ck p) b -> p ck b", p=P)
            BB = B_SH // NBLK
            for blk in range(NBLK):
                nc.scalar.dma_start(
                    out=yt_re[:, :, blk * BB:(blk + 1) * BB],
                    in_=yt_src[:, :, blk * BB:(blk + 1) * BB],
                )

            # resident natural y: y_sb[p, i, c] = y[i*128+p, c]
            y_sb = bigp.tile([P, NB * CP], BF16)
            y_re = y_sb.rearrange("p (i c) -> p i c", i=NB)
            y_src = y[:, :].rearrange("(i p) c -> p i c", p=P)
            TPB = NB // NBLK
            for blk in range(NBLK):
                nc.sync.dma_start(
                    out=y_re[:, blk * TPB:(blk + 1) * TPB, :],
                    in_=y_src[:, blk * TPB:(blk + 1) * TPB, :],
                )

            # colsum(H)[k] = sum_c H[c,k]  -> [1, C]
            cs_f = constp.tile([1, C], F32)
            for kn in range(KN):
                cs_ps = pzp.tile([1, KNS], F32, tag="z", name=f"cs_ps{kn}")
                for ck in range(CK):
                    nc.tensor.matmul(
                        cs_ps,
                        lhsT=ones_col,
                        rhs=h_sb[:, ck * C + kn * KNS: ck * C + (kn + 1) * KNS],
                        start=(ck == 0),
                        stop=(ck == CK - 1),
                    )
                nc.vector.tensor_copy(cs_f[:, kn * KNS:(kn + 1) * KNS], cs_ps)

            # write -colsum into H's zero padding rows 1022/1023 (chunk 7,
            # partitions 126/127) as bf16 value + bf16 residual
            cs_neg = constp.tile([1, C], F32)
            nc.vector.tensor_scalar_mul(cs_neg, cs_f, -1.0)
            r0_t = constp.tile([1, C], BF16)
            nc.vector.tensor_copy(r0_t, cs_neg)                   # r0 = bf16(-cs)
            res_f = constp.tile([1, C], F32)
            nc.vector.tensor_tensor(res_f, cs_neg, r0_t, op=SUB)  # -cs - r0
            res_t = constp.tile([1, C], BF16)
            nc.vector.tensor_copy(res_t, res_f)
            # engines can't address partitions 126/127 directly; DMA can
            nc.sync.dma_start(out=h_sb[P - 2:P - 1, (CK - 1) * C:CK * C], in_=r0_t)
            nc.sync.dma_start(out=h_sb[P - 1:P, (CK - 1) * C:CK * C], in_=res_t)

            # -1/sumH broadcast across partitions
            sum_h = constp.tile([1, 1], F32)
            nc.vector.reduce_sum(sum_h, cs_f, axis=mybir.AxisListType.X)
            recip_f = constp.tile([1, 1], F32)
            nc.vector.reciprocal(recip_f, sum_h)
            nrecip = constp.tile([1, 1], F32)
            nc.vector.tensor_scalar_mul(nrecip, recip_f, -1.0)
            recip_ps = prp.tile([P, 1], F32, name="recip_ps")
            nc.tensor.matmul(recip_ps, lhsT=ones_rf, rhs=nrecip, start=True, stop=True)
            nrecip_bc = constp.tile([P, 1], F32)
            nc.vector.tensor_copy(nrecip_bc, recip_ps)

            loss_acc = constp.tile([P, NB], F32)

            for i in range(NB):
                # PSUM <- ypad @ Hpad = Z - colsum_bcast
                s_half = []
                for kn in range(KN):
                    pz = pzp.tile([P, KNS], F32, tag="z", name="pz")
                    for ck in range(CK):
                        nc.tensor.matmul(
                            pz,
                            lhsT=yt_sb[:, ck * B_SH + i * P: ck * B_SH + (i + 1) * P],
                            rhs=h_sb[:, ck * C + kn * KNS: ck * C + (kn + 1) * KNS],
                            start=(ck == 0),
                            stop=(ck == CK - 1),
                        )
                    # s_kn = sum_k (-1/sumH) * (Z-colsum) * y   (fused)
                    s_o = accp.tile([P, 1], F32, name="s_o")
                    scr = scrp.tile([P, KNS], F32, name="scr")
                    nc.vector.scalar_tensor_tensor(
                        out=scr,
                        in0=pz,
                        scalar=nrecip_bc,
                        in1=y_sb[:, i * CP + kn * KNS: i * CP + (kn + 1) * KNS],
                        op0=MULT,
                        op1=MULT,
                        accum_out=s_o,
                    )
                    s_half.append(s_o)

                nc.vector.tensor_add(loss_acc[:, i:i + 1], s_half[0], s_half[1])

            nc.sync.dma_start(out=out[:, :], in_=loss_acc)

    if not nc.is_finalized():
        nc.finalize()
    return nc


def kernel(**inputs):
    global LAST_RESULTS
    y_pred = np.asarray(inputs["y_pred"])
    H = np.asarray(inputs["H"])
    assert y_pred.shape == (B, C) and H.shape == (C, C)

    # host-side layout/dtype prep: bf16 cast, contraction-dim zero-pad,
    # and a transposed copy for the matmul stationary operand.  The last
    # two padding columns of y are 1.0: they multiply the (-colsum) rows
    # the kernel writes into H's padding.
    y_b = np.zeros((B, CP), dtype=ml_dtypes.bfloat16)
    y_b[:, :C] = y_pred.astype(ml_dtypes.bfloat16)
    y_b[:, CP - 2:] = 1.0
    h_b = np.zeros((CP, C), dtype=ml_dtypes.bfloat16)
    h_b[:C, :] = H.astype(ml_dtypes.bfloat16)

    nc = _CACHE.get("nc")
    if nc is None:
        nc = _build()
        _CACHE["nc"] = nc

    in_maps = []
    for s in range(N_CORES):
        ys = y_b[s * B_SH:(s + 1) * B_SH]
        in_maps.append(
            {
                "y": np.ascontiguousarray(ys),
                "yt": np.ascontiguousarray(ys.T),
                "h": h_b,
            }
        )
    res = run_bass_kernel_spmd(
        nc,
        in_maps,
        core_ids=list(range(N_CORES)),
        trace=bool(int(os.environ.get("KBENCH_TRACE", "0"))),
    )
    LAST_RESULTS = res
    # loss_out is [128, 16] partition-major: element [p, i] = loss for shard
    # row i*128 + p. Transpose+flatten restores batch order per shard.
    loss = np.concatenate(
        [np.asarray(r["loss_out"]).T.reshape(-1) for r in res.results]
    ).astype(np.float32)
    return loss
